# revision 1
# baseline (speedup 1.0000x reference)
"""Trainium2 Bass kernel for DirectConv2D (3x3 VALID, NCHW/OIHW).

Problem: x [32, 256, 56, 56] int32 (values 0..7 after clip),
         weight [256, 256, 3, 3] fp32 (small non-negative ints 0..6)
         -> out [32, 256, 54, 54] fp32.

Strategy:
 - Data-parallel across 8 NeuronCores: 4 images per core, weight replicated.
 - Conv decomposed into 9 shifted matmuls (one per kernel tap) accumulated
   in PSUM; contraction over the 256 input channels.
 - Inputs are tiny non-negative integers, so fp8-e4m3 matmuls are exact
   (products <= 42, fp32 PSUM accumulation). DoubleRow perf mode contracts
   all 256 input channels (2 x 128-partition k-tiles) per matmul.
 - Activations live in SBUF as [128 part, chunk 2, img 4, pix 3140]
   (56*56=3136 pixels + 4 pad so every tile can read a full 504-wide
   window). Output computed in tiles of 9 rows x 56 cols = 504 <= 512
   (one PSUM bank); only the 54 valid cols per row are stored.
"""

import sys

sys.path.insert(0, "/opt/trn_rl_repo")

import ml_dtypes
import numpy as np

N_CORES = 8
IMGS = 4  # images per core
H = W = 56
OH = OW = 54
PIX = H * W  # 3136
PIXP = PIX + 4  # padded so kh=2,kw=2 window of width 504 stays in-bounds
ROWS_PER_TILE = 9
N_TILE = ROWS_PER_TILE * W  # 504 (<= 512 fp32 PSUM bank)
N_ROWTILES = OH // ROWS_PER_TILE  # 6

_PROGRAM_CACHE = {}


def _build_program(mode="fp8dr"):
    import concourse.bacc as bacc
    import concourse.mybir as mybir
    import concourse.tile as tile

    nc = bacc.Bacc(
        "TRN2",
        target_bir_lowering=False,
        debug=False,
        enable_asserts=False,
        num_devices=N_CORES,
    )
    dt8 = mybir.dt.float8e4
    dtb = mybir.dt.bfloat16
    dt_in = dt8 if mode == "fp8dr" else dtb

    x_d = nc.dram_tensor("x_sb", [128, 2, IMGS, PIXP], dt_in, kind="ExternalInput").ap()
    w_d = nc.dram_tensor("w_sb", [128, 2, 9, 2, 128], dt_in, kind="ExternalInput").ap()
    out_d = nc.dram_tensor(
        "out", [IMGS, 256, OH, OW], mybir.dt.float32, kind="ExternalOutput"
    ).ap()

    NT486 = ROWS_PER_TILE * OW  # 486 output pixels per row tile
    X0A_END = 1232  # image-0 leading tile: rows 0..21 (covers row tiles 0,1)
    X0M_BASE, X0M_END = 1008, 2140  # image-0 middle tile (row tiles 2,3)
    X0Z_BASE = 2016  # image-0 trailing tile (row tiles 4,5)

    with tile.TileContext(nc) as tc:
        with (
            tc.tile_pool(name="const", bufs=1) as const_pool,
            tc.tile_pool(name="psum", bufs=8, space="PSUM") as psum_pool,
            tc.tile_pool(name="outs", bufs=3) as out_pool,
        ):
            # PE warm-up on scratch: matmuls during the input-load window so
            # HAM un-throttles before the real stream starts. Results are
            # never read (next user of the PSUM slot starts with start=True),
            # so in the fp8 build the scratch stays uninitialized and the
            # warm-up has NO dependencies at all — it begins right after the
            # initial barrier. The sim'd bf16 build zeroes it for CoreSim.
            w_warm = const_pool.tile([128, 2, 128], dt_in)
            x_warm = const_pool.tile([128, 2, 544], dt_in)
            if mode != "fp8dr":
                nc.gpsimd.memset(w_warm, 0.0)
                nc.gpsimd.memset(x_warm, 0.0)
            else:
                # tiles must have a writer to be allocated; a 2-byte memset
                # is enough and keeps the warm-up dependency-free in practice
                nc.gpsimd.memset(w_warm[:, 0, 0:2], 0.0)
                nc.gpsimd.memset(x_warm[:, 0, 0:2], 0.0)
            pt_warm = psum_pool.tile([128, NT486], mybir.dt.float32, tag="pt")
            N_WARM = 13
            for i in range(N_WARM):
                rhs_w = x_warm[:, :, 0:N_TILE].rearrange(
                    "p c (r q) -> p c r q", q=W
                )[:, :, :, 0:OW]
                if mode == "fp8dr":
                    nc.tensor.matmul(
                        pt_warm, w_warm, rhs_w,
                        start=(i == 0), stop=(i == N_WARM - 1),
                        perf_mode=mybir.MatmulPerfMode.DoubleRow,
                    )
                else:
                    nc.tensor.matmul(
                        pt_warm, w_warm[:, 0], rhs_w[:, 0],
                        start=(i == 0), stop=(i == N_WARM - 1),
                    )

            wt = const_pool.tile([128, 2, 9, 2, 128], dt_in)
            # Per-image x tiles so matmul deps only cover the image they read
            # (dep tracking is per-tile). dma_start issue serializes ~0.6us
            # per engine sequencer and each ring FIFOs its transfers, so the
            # bytes gating the first matmul group (leading rows of image 0 +
            # first oc0 weight tap) go at the head of the sync ring; the rest
            # is ordered by first-use time across both rings.
            xt0a = const_pool.tile([128, 2, X0A_END], dt_in)
            xt0m = const_pool.tile([128, 2, X0M_END - X0M_BASE], dt_in)
            xt0z = const_pool.tile([128, 2, PIXP - X0Z_BASE], dt_in)
            xts = [None] + [
                const_pool.tile([128, 2, PIXP], dt_in, name=f"xt{n}", tag=f"xt{n}")
                for n in (1, 2, 3)
            ]
            # The first matmul group needs xt0a's leading chunks + the first
            # weight tap: split them across both rings so both completion
            # sems fire as early as possible. Everything else is ordered by
            # first-use time.
            # sync ring: image-0 lead (c0), weights, image 1
            nc.sync.dma_start(out=wt[:, 0, 0], in_=w_d[:, 0, 0])
            nc.sync.dma_start(out=xt0a[:, 0, 0:620], in_=x_d[:, 0, 0, 0:620])
            nc.sync.dma_start(out=wt[:, 0, 1:], in_=w_d[:, 0, 1:])
            nc.sync.dma_start(out=wt[:, 1], in_=w_d[:, 1])
            for c in range(2):
                nc.sync.dma_start(out=xts[1][:, c], in_=x_d[:, c, 1])
            # scalar ring: image-0 lead (c1), rest of image 0, images 2-3
            nc.scalar.dma_start(out=xt0a[:, 1, 0:620], in_=x_d[:, 1, 0, 0:620])
            for c in range(2):
                nc.scalar.dma_start(
                    out=xt0a[:, c, 620:], in_=x_d[:, c, 0, 620:X0A_END]
                )
            for c in range(2):
                nc.scalar.dma_start(
                    out=xt0m[:, c], in_=x_d[:, c, 0, X0M_BASE:X0M_END]
                )
            for c in range(2):
                nc.scalar.dma_start(out=xt0z[:, c], in_=x_d[:, c, 0, X0Z_BASE:])
            for n in (2, 3):
                for c in range(2):
                    nc.scalar.dma_start(out=xts[n][:, c], in_=x_d[:, c, n])

            def x_src(n, t):
                """(x tile, pixel base) holding rows needed by row tile t."""
                if n == 0:
                    if t < 2:
                        return xt0a, 0
                    if t < 4:
                        return xt0m, X0M_BASE
                    return xt0z, X0Z_BASE
                return xts[n], 0

            for n in range(IMGS):
                for oc in range(2):
                    # staging for a full (n, oc) output block: dense 54x54
                    # rows so stores move 11.7KB-contiguous lines/partition.
                    ot = out_pool.tile([128, OH * OW], mybir.dt.float32)
                    for t in range(N_ROWTILES):
                        h0 = t * ROWS_PER_TILE
                        xsrc, xbase = x_src(n, t)
                        pt = psum_pool.tile([128, NT486], mybir.dt.float32)
                        k = 0
                        for kh in range(3):
                            for kw in range(3):
                                off = (h0 + kh) * W + kw - xbase
                                # strided moving AP skips the 2 junk cols per
                                # row: [128, 2, 9 rows (stride 56), 54 cols]
                                if mode == "fp8dr":
                                    rhs = xsrc[:, :, off : off + N_TILE].rearrange(
                                        "p c (r q) -> p c r q", q=W
                                    )[:, :, :, 0:OW]
                                    nc.tensor.matmul(
                                        pt,
                                        wt[:, oc, k, :, :],
                                        rhs,
                                        start=(k == 0),
                                        stop=(k == 8),
                                        perf_mode=mybir.MatmulPerfMode.DoubleRow,
                                    )
                                else:
                                    for c in range(2):
                                        rhs = xsrc[:, c, off : off + N_TILE].rearrange(
                                            "p (r q) -> p r q", q=W
                                        )[:, :, 0:OW]
                                        nc.tensor.matmul(
                                            pt,
                                            wt[:, oc, k, c, :],
                                            rhs,
                                            start=(k == 0 and c == 0),
                                            stop=(k == 8 and c == 1),
                                        )
                                k += 1
                        last_block = n == IMGS - 1 and oc == 1
                        if last_block and t == N_ROWTILES - 1:
                            # very last tile: split the PSUM evacuation into
                            # two row-aligned halves so the copy and the two
                            # half-stores (on different rings) overlap.
                            s = 5 * OW  # 5 rows + 4 rows
                            base = t * NT486
                            nc.vector.tensor_copy(
                                out=ot[:, base : base + s], in_=pt[:, 0:s]
                            )
                            nc.sync.dma_start(
                                out=out_d[n, oc * 128 : (oc + 1) * 128,
                                          h0 : h0 + 5, :],
                                in_=ot[:, base : base + s].rearrange(
                                    "p (h w) -> p h w", w=OW
                                ),
                            )
                            nc.vector.tensor_copy(
                                out=ot[:, base + s : base + NT486],
                                in_=pt[:, s:NT486],
                            )
                            nc.scalar.dma_start(
                                out=out_d[n, oc * 128 : (oc + 1) * 128,
                                          h0 + 5 : h0 + ROWS_PER_TILE, :],
                                in_=ot[:, base + s : base + NT486].rearrange(
                                    "p (h w) -> p h w", w=OW
                                ),
                            )
                        else:
                            nc.vector.tensor_copy(
                                out=ot[:, t * NT486 : (t + 1) * NT486], in_=pt
                            )
                        if last_block:
                            # fine-grained stores on the final block: pairs
                            # early, singles at the end so the final store
                            # (and its completion latency) is small.
                            if t in (1, 3):
                                nc.sync.dma_start(
                                    out=out_d[n, oc * 128 : (oc + 1) * 128,
                                              h0 - ROWS_PER_TILE : h0 + ROWS_PER_TILE, :],
                                    in_=ot[:, (t - 1) * NT486 : (t + 1) * NT486].rearrange(
                                        "p (h w) -> p h w", w=OW
                                    ),
                                )
                            elif t == 4:  # t=5 stores in halves above
                                nc.sync.dma_start(
                                    out=out_d[n, oc * 128 : (oc + 1) * 128,
                                              h0 : h0 + ROWS_PER_TILE, :],
                                    in_=ot[:, t * NT486 : (t + 1) * NT486].rearrange(
                                        "p (h w) -> p h w", w=OW
                                    ),
                                )
                    if not last_block:
                        nc.sync.dma_start(
                            out=out_d[n, oc * 128 : (oc + 1) * 128, :, :],
                            in_=ot.rearrange("p (h w) -> p h w", w=OW),
                        )
    nc.compile()
    return nc


def get_program(mode="fp8dr"):
    if mode not in _PROGRAM_CACHE:
        _PROGRAM_CACHE[mode] = _build_program(mode)
    return _PROGRAM_CACHE[mode]


def _np_dtype(mode):
    return ml_dtypes.float8_e4m3 if mode == "fp8dr" else ml_dtypes.bfloat16


def prep_weight(weight, mode="fp8dr"):
    """weight [256, 256, 3, 3] OIHW fp32 -> w_sb [128 ki, 2 oc, 9 tap, 2 c, 128 m]."""
    wq = weight.astype(np.int32).astype(np.float32)
    wq = wq.reshape(2, 128, 2, 128, 3, 3)  # [oc, m, c, ki, kh, kw]
    w_sb = np.ascontiguousarray(wq.transpose(3, 0, 4, 5, 2, 1))  # [ki, oc, kh, kw, c, m]
    w_sb = w_sb.reshape(128, 2, 9, 2, 128)
    return w_sb.astype(_np_dtype(mode))


def prep_x_core(x_core, mode="fp8dr"):
    """x_core [IMGS, 256, 56, 56] int32 -> x_sb [128 ki, 2 c, IMGS, PIXP]."""
    xq = np.clip(x_core.astype(np.int32), 0, 7).astype(np.float32)
    xq = xq.reshape(IMGS, 2, 128, PIX)  # [n, c, ki, pix]
    x_sb = np.zeros((128, 2, IMGS, PIXP), np.float32)
    x_sb[:, :, :, :PIX] = xq.transpose(2, 1, 0, 3)
    return x_sb.astype(_np_dtype(mode))


def make_in_maps(x, weight, mode="fp8dr"):
    w_sb = prep_weight(weight, mode)
    return [
        {"x_sb": prep_x_core(x[c * IMGS : (c + 1) * IMGS], mode), "w_sb": w_sb}
        for c in range(N_CORES)
    ]


def kernel(x, weight):
    import time

    from concourse.bass_utils import run_bass_kernel_spmd

    mode = "fp8dr"
    nc = get_program(mode)
    in_maps = make_in_maps(np.asarray(x), np.asarray(weight), mode)
    last_err = None
    for attempt in range(3):
        try:
            res = run_bass_kernel_spmd(nc, in_maps, list(range(N_CORES)))
            break
        except Exception as e:  # transient NRT_EXEC_UNIT_UNRECOVERABLE flakes
            last_err = e
            time.sleep(2.0)
    else:
        raise last_err
    return np.concatenate(
        [res.results[c]["out"] for c in range(N_CORES)], axis=0
    ).astype(np.float32)



# revision 3
# speedup vs baseline: 1.0111x; 1.0111x over previous
"""Trainium2 Bass kernel for DirectConv2D (3x3 VALID, NCHW/OIHW).

Problem: x [32, 256, 56, 56] int32 (values 0..7 after clip),
         weight [256, 256, 3, 3] fp32 (small non-negative ints 0..6)
         -> out [32, 256, 54, 54] fp32.

Strategy (v2): 1D Winograd F(2,3) along W + direct 3-tap conv along H.
 - Data-parallel across 8 NeuronCores: 4 images per core, weight replicated.
 - Host precomputes the input transform d[pos] (4 planes of ints in [-14,14],
   exact in fp8) and the weight transform w1[kh,pos] = G @ w (halves in
   [-3, 9]; only the value 8.5 rounds in e4m3 -> max rel err ~5e-3 measured
   against the reference on the real data, well under the 2e-2 gate).
 - M[pos] = sum_kh W1[kh,pos]^T @ d[pos](rows r+kh): fp8 DoubleRow matmuls
   (256-channel contraction per pass), 3-tap PSUM accumulation.
   Per core: 4 img x 2 oc x 3 row-chunks x 4 pos x 3 kh = 288 matmuls of
   486 moving cols (vs 432 for direct conv: 1.5x fewer PE cycles).
 - Output transform on-chip: out_even = M0+M1+M2, out_odd = M1-M2-M3,
   computed with ACT copies (PSUM->SBUF) + DVE tensor_tensor ops (each DVE
   op reads at most ONE PSUM operand - PSUM has a single DVE read port),
   writing bf16 interleaved columns directly (abs err <= ~137 vs outputs
   >= 21k). Host upcasts the bf16 result to fp32.
"""

import sys

sys.path.insert(0, "/opt/trn_rl_repo")

import ml_dtypes
import numpy as np

N_CORES = 8
IMGS = 4  # images per core
H = W = 56
OH = OW = 54
TC = 27  # col tiles (2 output cols each)
ROWS_PER_CHUNK = 18
N_CHUNKS = OH // ROWS_PER_CHUNK  # 3
NT = ROWS_PER_CHUNK * TC  # 486 (<= 512 fp32 PSUM bank)
DPIX = H * TC  # 1512 per (c, pos, img) plane

_PROGRAM_CACHE = {}


def _build_program(mode="fp8dr"):
    import concourse.bacc as bacc
    import concourse.mybir as mybir
    import concourse.tile as tile

    nc = bacc.Bacc(
        "TRN2",
        target_bir_lowering=False,
        debug=False,
        enable_asserts=False,
        num_devices=N_CORES,
    )
    dt8 = mybir.dt.float8e4
    dtb = mybir.dt.bfloat16
    dt_in = dt8 if mode == "fp8dr" else dtb
    fp32 = mybir.dt.float32

    # d planes: [ki, c, pos, img, h*tc]
    d_d = nc.dram_tensor("x_sb", [128, 2, 4, IMGS, DPIX], dt_in, kind="ExternalInput").ap()
    # transformed weights: [ki, kh, pos, oc, c, m]
    w_d = nc.dram_tensor("w_sb", [128, 3, 4, 2, 2, 128], dt_in, kind="ExternalInput").ap()
    out_d = nc.dram_tensor(
        "out", [IMGS, 256, OH, OW], dtb, kind="ExternalOutput"
    ).ap()

    with tile.TileContext(nc) as tc:
        with (
            tc.tile_pool(name="const", bufs=1) as const_pool,
            tc.tile_pool(name="psum", bufs=8, space="PSUM") as psum_pool,
            tc.tile_pool(name="tmp", bufs=8) as tmp_pool,
            tc.tile_pool(name="outs", bufs=4) as out_pool,
        ):
            # PE warm-up on scratch during the input-load window (HAM
            # un-throttle). fp8 build: minimal memset so the warm-up has no
            # real dependencies; bf16 build zeroes for CoreSim.
            w_warm = const_pool.tile([128, 2, 128], dt_in)
            x_warm = const_pool.tile([128, 2, 544], dt_in)
            if mode != "fp8dr":
                nc.gpsimd.memset(w_warm, 0.0)
                nc.gpsimd.memset(x_warm, 0.0)
            else:
                nc.gpsimd.memset(w_warm[:, 0, 0:2], 0.0)
                nc.gpsimd.memset(x_warm[:, 0, 0:2], 0.0)
            pt_warm = psum_pool.tile([128, NT], fp32, tag="pt")
            N_WARM = 13
            for i in range(N_WARM):
                rhs_w = x_warm[:, :, 0:NT].rearrange("p c (r q) -> p c r q", q=TC)
                if mode == "fp8dr":
                    nc.tensor.matmul(
                        pt_warm, w_warm, rhs_w,
                        start=(i == 0), stop=(i == N_WARM - 1),
                        perf_mode=mybir.MatmulPerfMode.DoubleRow,
                    )
                else:
                    nc.tensor.matmul(
                        pt_warm, w_warm[:, 0], rhs_w[:, 0],
                        start=(i == 0), stop=(i == N_WARM - 1),
                    )

            wt = const_pool.tile([128, 3, 4, 2, 2, 128], dt_in)
            # per-image d tiles: [ki, c, pos, h*tc]
            dts = [
                const_pool.tile([128, 2, 4, DPIX], dt_in, name=f"dt{n}", tag=f"dt{n}")
                for n in range(IMGS)
            ]
            # First MM group needs: wt oc0 (all kh/pos) + dt0 rows 0..19.
            # Split the critical bytes across both rings, then order the rest
            # by first use.
            LEAD = 20 * TC  # rows 0..19
            nc.sync.dma_start(out=wt[:, :, :, 0], in_=w_d[:, :, :, 0])
            nc.sync.dma_start(out=dts[0][:, 0, :, 0:LEAD], in_=d_d[:, 0, :, 0, 0:LEAD])
            nc.scalar.dma_start(out=dts[0][:, 1, :, 0:LEAD], in_=d_d[:, 1, :, 0, 0:LEAD])
            nc.scalar.dma_start(out=wt[:, :, :, 1], in_=w_d[:, :, :, 1])
            for c in range(2):
                nc.sync.dma_start(
                    out=dts[0][:, c, :, LEAD:], in_=d_d[:, c, :, 0, LEAD:]
                )
            # remaining images: interleave rings, ordered by first use
            for n in range(1, IMGS):
                nc.sync.dma_start(out=dts[n][:, 0], in_=d_d[:, 0, :, n])
                nc.scalar.dma_start(out=dts[n][:, 1], in_=d_d[:, 1, :, n])

            n_group = 0
            N_GROUPS = IMGS * 2 * N_CHUNKS
            for n in range(IMGS):
                for oc in range(2):
                    for ch in range(N_CHUNKS):
                        h0 = ch * ROWS_PER_CHUNK
                        pts = []
                        for pos in range(4):
                            pt = psum_pool.tile([128, NT], fp32)
                            pts.append(pt)
                            for kh in range(3):
                                off = (h0 + kh) * TC
                                if mode == "fp8dr":
                                    rhs = dts[n][:, :, pos, off : off + NT].rearrange(
                                        "p c (r q) -> p c r q", q=TC
                                    )
                                    nc.tensor.matmul(
                                        pt,
                                        wt[:, kh, pos, oc],
                                        rhs,
                                        start=(kh == 0),
                                        stop=(kh == 2),
                                        perf_mode=mybir.MatmulPerfMode.DoubleRow,
                                    )
                                else:
                                    for c in range(2):
                                        rhs = dts[n][:, c, pos, off : off + NT].rearrange(
                                            "p (r q) -> p r q", q=TC
                                        )
                                        nc.tensor.matmul(
                                            pt,
                                            wt[:, kh, pos, oc, c],
                                            rhs,
                                            start=(kh == 0 and c == 0),
                                            stop=(kh == 2 and c == 1),
                                        )
                        # output transform: even cols = M0+M1+M2,
                        # odd cols = M1-M2-M3. ACT stages M0 and M2 into
                        # SBUF; every DVE op reads exactly one PSUM tile.
                        ot = out_pool.tile([128, ROWS_PER_CHUNK, OW], dtb)
                        t0 = tmp_pool.tile([128, NT], fp32)
                        t2 = tmp_pool.tile([128, NT], fp32)
                        a = tmp_pool.tile([128, NT], fp32)
                        b = tmp_pool.tile([128, NT], fp32)
                        nc.scalar.copy(t0, pts[0])
                        nc.scalar.copy(t2, pts[2])
                        nc.vector.tensor_tensor(a, pts[1], t0, mybir.AluOpType.add)
                        nc.vector.tensor_tensor(
                            ot[:, :, 0::2].rearrange("p h w -> p (h w)"),
                            pts[2], a, mybir.AluOpType.add,
                        )
                        nc.vector.tensor_tensor(b, pts[1], t2, mybir.AluOpType.subtract)
                        nc.vector.tensor_tensor(
                            ot[:, :, 1::2].rearrange("p h w -> p (h w)"),
                            b, pts[3], mybir.AluOpType.subtract,
                        )
                        n_group += 1
                        last = n_group == N_GROUPS
                        if last:
                            # split the final store across both rings so the
                            # completion tail is short
                            s = ROWS_PER_CHUNK // 2
                            nc.sync.dma_start(
                                out=out_d[n, oc * 128 : (oc + 1) * 128, h0 : h0 + s, :],
                                in_=ot[:, 0:s, :],
                            )
                            nc.scalar.dma_start(
                                out=out_d[n, oc * 128 : (oc + 1) * 128,
                                          h0 + s : h0 + ROWS_PER_CHUNK, :],
                                in_=ot[:, s:, :],
                            )
                        else:
                            ring = nc.sync if (n_group % 2 == 0) else nc.scalar
                            ring.dma_start(
                                out=out_d[n, oc * 128 : (oc + 1) * 128,
                                          h0 : h0 + ROWS_PER_CHUNK, :],
                                in_=ot,
                            )
    nc.compile()
    return nc


def get_program(mode="fp8dr"):
    if mode not in _PROGRAM_CACHE:
        _PROGRAM_CACHE[mode] = _build_program(mode)
    return _PROGRAM_CACHE[mode]


def _np_dtype(mode):
    return ml_dtypes.float8_e4m3 if mode == "fp8dr" else ml_dtypes.bfloat16


def prep_weight(weight, mode="fp8dr"):
    """weight [256,256,3,3] OIHW fp32 -> w_sb [128 ki, 3 kh, 4 pos, 2 oc, 2 c, 128 m].

    w1[o,i,kh,pos] = sum_kw G[pos,kw] w[o,i,kh,kw], G = F(2,3) weight transform.
    """
    G = np.array([[1, 0, 0], [0.5, 0.5, 0.5], [0.5, -0.5, 0.5], [0, 0, 1]], np.float32)
    wq = weight.astype(np.int32).astype(np.float32)
    w1 = np.einsum("pk,oihk->oihp", G, wq)  # [o, i, kh, pos]
    w1 = w1.reshape(2, 128, 2, 128, 3, 4)  # [oc, m, c, ki, kh, pos]
    w_sb = np.ascontiguousarray(w1.transpose(3, 4, 5, 0, 2, 1))  # [ki, kh, pos, oc, c, m]
    return w_sb.astype(_np_dtype(mode))


def prep_x_core(x_core, mode="fp8dr"):
    """x_core [IMGS, 256, 56, 56] int32 -> d_sb [128 ki, 2 c, 4 pos, IMGS, 56*27]."""
    xq = np.clip(x_core.astype(np.int32), 0, 7).astype(np.float32)
    xq = xq.reshape(IMGS, 2, 128, H, W)  # [n, c, ki, h, w]
    d0 = xq[..., 0:54:2] - xq[..., 2:56:2]
    d1 = xq[..., 1:55:2] + xq[..., 2:56:2]
    d2 = xq[..., 2:56:2] - xq[..., 1:55:2]
    d3 = xq[..., 1:55:2] - xq[..., 3:56:2]
    d = np.stack([d0, d1, d2, d3], axis=0)  # [pos, n, c, ki, h, tc]
    d_sb = np.ascontiguousarray(d.transpose(3, 2, 0, 1, 4, 5))  # [ki, c, pos, n, h, tc]
    return d_sb.reshape(128, 2, 4, IMGS, DPIX).astype(_np_dtype(mode))


def make_in_maps(x, weight, mode="fp8dr"):
    w_sb = prep_weight(weight, mode)
    return [
        {"x_sb": prep_x_core(x[c * IMGS : (c + 1) * IMGS], mode), "w_sb": w_sb}
        for c in range(N_CORES)
    ]


def kernel(x, weight):
    import time

    from concourse.bass_utils import run_bass_kernel_spmd

    mode = "fp8dr"
    nc = get_program(mode)
    in_maps = make_in_maps(np.asarray(x), np.asarray(weight), mode)
    last_err = None
    for attempt in range(3):
        try:
            res = run_bass_kernel_spmd(nc, in_maps, list(range(N_CORES)))
            break
        except Exception as e:  # transient NRT_EXEC_UNIT_UNRECOVERABLE flakes
            last_err = e
            time.sleep(2.0)
    else:
        raise last_err
    return np.concatenate(
        [res.results[c]["out"] for c in range(N_CORES)], axis=0
    ).astype(np.float32)


# revision 6
# speedup vs baseline: 1.0688x; 1.0571x over previous
"""Trainium2 Bass kernel for DirectConv2D (3x3 VALID, NCHW/OIHW).

Problem: x [32, 256, 56, 56] int32 (values 0..7 after clip),
         weight [256, 256, 3, 3] fp32 (small non-negative ints 0..6)
         -> out [32, 256, 54, 54] fp32.

Strategy (v2): 1D Winograd F(2,3) along W + direct 3-tap conv along H.
 - Data-parallel across 8 NeuronCores: 4 images per core, weight replicated.
 - Host precomputes the input transform d[pos] (4 planes of ints in [-14,14],
   exact in fp8) and the weight transform w1[kh,pos] = G @ w (halves in
   [-3, 9]; only the value 8.5 rounds in e4m3 -> max rel err ~5e-3 measured
   against the reference on the real data, well under the 2e-2 gate).
 - M[pos] = sum_kh W1[kh,pos]^T @ d[pos](rows r+kh): fp8 DoubleRow matmuls
   (256-channel contraction per pass), 3-tap PSUM accumulation.
   Per core: 4 img x 2 oc x 3 row-chunks x 4 pos x 3 kh = 288 matmuls of
   486 moving cols (vs 432 for direct conv: 1.5x fewer PE cycles).
 - Output transform on-chip: out_even = M0+M1+M2, out_odd = M1-M2-M3,
   computed with ACT copies (PSUM->SBUF) + DVE tensor_tensor ops (each DVE
   op reads at most ONE PSUM operand - PSUM has a single DVE read port),
   writing bf16 interleaved columns directly (abs err <= ~137 vs outputs
   >= 21k). Host upcasts the bf16 result to fp32.
"""

import sys

sys.path.insert(0, "/opt/trn_rl_repo")

import ml_dtypes
import numpy as np

N_CORES = 8
IMGS = 4  # images per core
H = W = 56
OH = OW = 54
TC = 27  # col tiles (2 output cols each)
ROWS_PER_CHUNK = 18
N_CHUNKS = OH // ROWS_PER_CHUNK  # 3
NT = ROWS_PER_CHUNK * TC  # 486 (<= 512 fp32 PSUM bank)
DPIX = H * TC  # 1512 per (c, pos, img) plane

_PROGRAM_CACHE = {}


def _build_program(mode="fp8dr"):
    import concourse.bacc as bacc
    import concourse.mybir as mybir
    import concourse.tile as tile

    nc = bacc.Bacc(
        "TRN2",
        target_bir_lowering=False,
        debug=False,
        enable_asserts=False,
        num_devices=N_CORES,
    )
    dt8 = mybir.dt.float8e4
    dtb = mybir.dt.bfloat16
    dt_in = dt8 if mode == "fp8dr" else dtb
    fp32 = mybir.dt.float32

    # d planes: [ki, c, pos, img, h*tc]
    d_d = nc.dram_tensor("x_sb", [128, 2, 4, IMGS, DPIX], dt_in, kind="ExternalInput").ap()
    # transformed weights: [ki, kh, pos, oc, c, m]
    w_d = nc.dram_tensor("w_sb", [128, 3, 4, 2, 2, 128], dt_in, kind="ExternalInput").ap()
    out_d = nc.dram_tensor(
        "out", [IMGS, 256, OH, OW], dtb, kind="ExternalOutput"
    ).ap()

    with tile.TileContext(nc) as tc:
        with (
            tc.tile_pool(name="const", bufs=1) as const_pool,
            tc.tile_pool(name="psum", bufs=8, space="PSUM") as psum_pool,
            tc.tile_pool(name="tmp", bufs=8) as tmp_pool,
            tc.tile_pool(name="outs", bufs=4) as out_pool,
        ):
            # PE warm-up on scratch during the input-load window (HAM
            # un-throttle). fp8 build: minimal memset so the warm-up has no
            # real dependencies; bf16 build zeroes for CoreSim.
            w_warm = const_pool.tile([128, 2, 128], dt_in)
            x_warm = const_pool.tile([128, 2, 544], dt_in)
            if mode != "fp8dr":
                nc.gpsimd.memset(w_warm, 0.0)
                nc.gpsimd.memset(x_warm, 0.0)
            else:
                nc.gpsimd.memset(w_warm[:, 0, 0:2], 0.0)
                nc.gpsimd.memset(x_warm[:, 0, 0:2], 0.0)
            pt_warm = psum_pool.tile([128, NT], fp32, tag="pt")
            N_WARM = 13
            for i in range(N_WARM):
                rhs_w = x_warm[:, :, 0:NT]
                if mode == "fp8dr":
                    nc.tensor.matmul(
                        pt_warm, w_warm, rhs_w,
                        start=(i == 0), stop=(i == N_WARM - 1),
                        perf_mode=mybir.MatmulPerfMode.DoubleRow,
                    )
                else:
                    nc.tensor.matmul(
                        pt_warm, w_warm[:, 0], rhs_w[:, 0],
                        start=(i == 0), stop=(i == N_WARM - 1),
                    )

            wt = const_pool.tile([128, 3, 4, 2, 2, 128], dt_in)
            # per-image d tiles: [ki, c, pos, h*tc]
            dts = [
                const_pool.tile([128, 2, 4, DPIX], dt_in, name=f"dt{n}", tag=f"dt{n}")
                for n in range(IMGS)
            ]
            # First MM group needs: wt oc0 (all kh/pos) + dt0 rows 0..19.
            # Split the critical bytes across both rings, then order the rest
            # by first use.
            LEAD = 20 * TC  # rows 0..19
            nc.sync.dma_start(out=wt[:, :, :, 0], in_=w_d[:, :, :, 0])
            nc.sync.dma_start(out=dts[0][:, 0, :, 0:LEAD], in_=d_d[:, 0, :, 0, 0:LEAD])
            nc.scalar.dma_start(out=dts[0][:, 1, :, 0:LEAD], in_=d_d[:, 1, :, 0, 0:LEAD])
            nc.scalar.dma_start(out=wt[:, :, :, 1], in_=w_d[:, :, :, 1])
            for c in range(2):
                nc.sync.dma_start(
                    out=dts[0][:, c, :, LEAD:], in_=d_d[:, c, :, 0, LEAD:]
                )
            # remaining images: interleave rings, ordered by first use
            for n in range(1, IMGS):
                nc.sync.dma_start(out=dts[n][:, 0], in_=d_d[:, 0, :, n])
                nc.scalar.dma_start(out=dts[n][:, 1], in_=d_d[:, 1, :, n])

            n_group = 0
            N_GROUPS = IMGS * 2 * N_CHUNKS
            for n in range(IMGS):
                for oc in range(2):
                    for ch in range(N_CHUNKS):
                        h0 = ch * ROWS_PER_CHUNK
                        pts = []
                        for pos in range(4):
                            pt = psum_pool.tile([128, NT], fp32)
                            pts.append(pt)
                            for kh in range(3):
                                off = (h0 + kh) * TC
                                # winograd rhs windows are fully contiguous:
                                # flat 486-elem inner dim streams best
                                if mode == "fp8dr":
                                    rhs = dts[n][:, :, pos, off : off + NT]
                                    nc.tensor.matmul(
                                        pt,
                                        wt[:, kh, pos, oc],
                                        rhs,
                                        start=(kh == 0),
                                        stop=(kh == 2),
                                        perf_mode=mybir.MatmulPerfMode.DoubleRow,
                                    )
                                else:
                                    for c in range(2):
                                        rhs = dts[n][:, c, pos, off : off + NT]
                                        nc.tensor.matmul(
                                            pt,
                                            wt[:, kh, pos, oc, c],
                                            rhs,
                                            start=(kh == 0 and c == 0),
                                            stop=(kh == 2 and c == 1),
                                        )
                        # output transform: even cols = M0+M1+M2,
                        # odd cols = M1-M2-M3. PSUM has one DVE read port,
                        # so ACT stages M1/M2 into SBUF, GPSIMD (no PSUM
                        # access) takes the SBUF-only subtract, and each
                        # DVE op reads exactly one PSUM tile.
                        ot = out_pool.tile([128, ROWS_PER_CHUNK, OW], dtb)
                        t1 = tmp_pool.tile([128, NT], fp32)
                        t2 = tmp_pool.tile([128, NT], fp32)
                        a = tmp_pool.tile([128, NT], fp32)
                        s = tmp_pool.tile([128, NT], fp32)
                        nc.scalar.copy(t1, pts[1])
                        nc.scalar.copy(t2, pts[2])
                        nc.gpsimd.tensor_tensor(s, t1, t2, mybir.AluOpType.subtract)
                        nc.vector.tensor_tensor(a, pts[0], t1, mybir.AluOpType.add)
                        nc.vector.tensor_tensor(
                            ot[:, :, 0::2].rearrange("p h w -> p (h w)"),
                            a, t2, mybir.AluOpType.add,
                        )
                        nc.vector.tensor_tensor(
                            ot[:, :, 1::2].rearrange("p h w -> p (h w)"),
                            s, pts[3], mybir.AluOpType.subtract,
                        )
                        n_group += 1
                        last = n_group == N_GROUPS
                        if last:
                            # split the final store across both rings so the
                            # completion tail is short
                            s = ROWS_PER_CHUNK // 2
                            nc.sync.dma_start(
                                out=out_d[n, oc * 128 : (oc + 1) * 128, h0 : h0 + s, :],
                                in_=ot[:, 0:s, :],
                            )
                            nc.scalar.dma_start(
                                out=out_d[n, oc * 128 : (oc + 1) * 128,
                                          h0 + s : h0 + ROWS_PER_CHUNK, :],
                                in_=ot[:, s:, :],
                            )
                        else:
                            ring = nc.sync if (n_group % 2 == 0) else nc.scalar
                            ring.dma_start(
                                out=out_d[n, oc * 128 : (oc + 1) * 128,
                                          h0 : h0 + ROWS_PER_CHUNK, :],
                                in_=ot,
                            )
    nc.compile()
    return nc


def get_program(mode="fp8dr"):
    if mode not in _PROGRAM_CACHE:
        _PROGRAM_CACHE[mode] = _build_program(mode)
    return _PROGRAM_CACHE[mode]


def _np_dtype(mode):
    return ml_dtypes.float8_e4m3 if mode == "fp8dr" else ml_dtypes.bfloat16


def prep_weight(weight, mode="fp8dr"):
    """weight [256,256,3,3] OIHW fp32 -> w_sb [128 ki, 3 kh, 4 pos, 2 oc, 2 c, 128 m].

    w1[o,i,kh,pos] = sum_kw G[pos,kw] w[o,i,kh,kw], G = F(2,3) weight transform.
    """
    G = np.array([[1, 0, 0], [0.5, 0.5, 0.5], [0.5, -0.5, 0.5], [0, 0, 1]], np.float32)
    wq = weight.astype(np.int32).astype(np.float32)
    w1 = np.einsum("pk,oihk->oihp", G, wq)  # [o, i, kh, pos]
    w1 = w1.reshape(2, 128, 2, 128, 3, 4)  # [oc, m, c, ki, kh, pos]
    w_sb = np.ascontiguousarray(w1.transpose(3, 4, 5, 0, 2, 1))  # [ki, kh, pos, oc, c, m]
    return w_sb.astype(_np_dtype(mode))


def prep_x_core(x_core, mode="fp8dr"):
    """x_core [IMGS, 256, 56, 56] int32 -> d_sb [128 ki, 2 c, 4 pos, IMGS, 56*27]."""
    xq = np.clip(x_core.astype(np.int32), 0, 7).astype(np.float32)
    xq = xq.reshape(IMGS, 2, 128, H, W)  # [n, c, ki, h, w]
    d0 = xq[..., 0:54:2] - xq[..., 2:56:2]
    d1 = xq[..., 1:55:2] + xq[..., 2:56:2]
    d2 = xq[..., 2:56:2] - xq[..., 1:55:2]
    d3 = xq[..., 1:55:2] - xq[..., 3:56:2]
    d = np.stack([d0, d1, d2, d3], axis=0)  # [pos, n, c, ki, h, tc]
    d_sb = np.ascontiguousarray(d.transpose(3, 2, 0, 1, 4, 5))  # [ki, c, pos, n, h, tc]
    return d_sb.reshape(128, 2, 4, IMGS, DPIX).astype(_np_dtype(mode))


def make_in_maps(x, weight, mode="fp8dr"):
    w_sb = prep_weight(weight, mode)
    return [
        {"x_sb": prep_x_core(x[c * IMGS : (c + 1) * IMGS], mode), "w_sb": w_sb}
        for c in range(N_CORES)
    ]


def kernel(x, weight):
    import time

    from concourse.bass_utils import run_bass_kernel_spmd

    mode = "fp8dr"
    nc = get_program(mode)
    in_maps = make_in_maps(np.asarray(x), np.asarray(weight), mode)
    last_err = None
    for attempt in range(3):
        try:
            res = run_bass_kernel_spmd(nc, in_maps, list(range(N_CORES)))
            break
        except Exception as e:  # transient NRT_EXEC_UNIT_UNRECOVERABLE flakes
            last_err = e
            time.sleep(2.0)
    else:
        raise last_err
    return np.concatenate(
        [res.results[c]["out"] for c in range(N_CORES)], axis=0
    ).astype(np.float32)


# revision 8
# speedup vs baseline: 1.2122x; 1.1341x over previous
"""Trainium2 Bass kernel for DirectConv2D (3x3 VALID, NCHW/OIHW).

Problem: x [32, 256, 56, 56] int32 (values 0..7 after clip),
         weight [256, 256, 3, 3] fp32 (small non-negative ints 0..6)
         -> out [32, 256, 54, 54] fp32.

Strategy (v2): 1D Winograd F(2,3) along W + direct 3-tap conv along H.
 - Data-parallel across 8 NeuronCores: 4 images per core, weight replicated.
 - Host precomputes the input transform d[pos] (4 planes of ints in [-14,14],
   exact in fp8) and the weight transform w1[kh,pos] = G @ w (halves in
   [-3, 9]; only the value 8.5 rounds in e4m3 -> max rel err ~5e-3 measured
   against the reference on the real data, well under the 2e-2 gate).
 - M[pos] = sum_kh W1[kh,pos]^T @ d[pos](rows r+kh): fp8 DoubleRow matmuls
   (256-channel contraction per pass), 3-tap PSUM accumulation.
   Per core: 4 img x 2 oc x 3 row-chunks x 4 pos x 3 kh = 288 matmuls of
   486 moving cols (vs 432 for direct conv: 1.5x fewer PE cycles).
 - Output transform on-chip: out_even = M0+M1+M2, out_odd = M1-M2-M3,
   computed with ACT copies (PSUM->SBUF) + DVE tensor_tensor ops (each DVE
   op reads at most ONE PSUM operand - PSUM has a single DVE read port),
   writing bf16 interleaved columns directly (abs err <= ~137 vs outputs
   >= 21k). Host upcasts the bf16 result to fp32.
"""

import sys

sys.path.insert(0, "/opt/trn_rl_repo")

import ml_dtypes
import numpy as np

N_CORES = 8
IMGS = 4  # images per core
H = W = 56
OH = OW = 54
TC = 27  # col tiles (2 output cols each)
ROWS_PER_CHUNK = 18
N_CHUNKS = OH // ROWS_PER_CHUNK  # 3
NT = ROWS_PER_CHUNK * TC  # 486 (<= 512 fp32 PSUM bank)
DPIX = H * TC  # 1512 per (c, pos, img) plane

_PROGRAM_CACHE = {}


def _build_program(mode="fp8dr"):
    import concourse.bacc as bacc
    import concourse.mybir as mybir
    import concourse.tile as tile

    nc = bacc.Bacc(
        "TRN2",
        target_bir_lowering=False,
        debug=False,
        enable_asserts=False,
        num_devices=N_CORES,
    )
    dt8 = mybir.dt.float8e4
    dtb = mybir.dt.bfloat16
    dt_in = dt8 if mode == "fp8dr" else dtb
    fp32 = mybir.dt.float32

    # d planes: [ki, c, pos, img, h*tc]
    d_d = nc.dram_tensor("x_sb", [128, 2, 4, IMGS, DPIX], dt_in, kind="ExternalInput").ap()
    # transformed weights: [ki, kh, pos, oc, c, m]
    w_d = nc.dram_tensor("w_sb", [128, 3, 4, 2, 2, 128], dt_in, kind="ExternalInput").ap()
    out_d = nc.dram_tensor(
        "out", [IMGS, 256, OH, OW], dtb, kind="ExternalOutput"
    ).ap()

    with tile.TileContext(nc) as tc:
        with (
            tc.tile_pool(name="const", bufs=1) as const_pool,
            tc.tile_pool(name="psum", bufs=8, space="PSUM") as psum_pool,
            tc.tile_pool(name="tmp", bufs=8) as tmp_pool,
            tc.tile_pool(name="outs", bufs=4) as out_pool,
        ):
            # PE warm-up on scratch during the input-load window (HAM
            # un-throttle). fp8 build: minimal memset so the warm-up has no
            # real dependencies; bf16 build zeroes for CoreSim.
            w_warm = const_pool.tile([128, 2, 128], dt_in)
            x_warm = const_pool.tile([128, 2, 544], dt_in)
            if mode != "fp8dr":
                nc.gpsimd.memset(w_warm, 0.0)
                nc.gpsimd.memset(x_warm, 0.0)
            else:
                nc.gpsimd.memset(w_warm[:, 0, 0:2], 0.0)
                nc.gpsimd.memset(x_warm[:, 0, 0:2], 0.0)
            pt_warm = psum_pool.tile([128, NT], fp32, tag="pt")
            # just enough warm-up to keep PE busy until the first input DMA
            # lands; the real MM stream then keeps HAM activity continuous
            N_WARM = 4
            for i in range(N_WARM):
                rhs_w = x_warm[:, :, 0:NT]
                if mode == "fp8dr":
                    nc.tensor.matmul(
                        pt_warm, w_warm, rhs_w,
                        start=(i == 0), stop=(i == N_WARM - 1),
                        perf_mode=mybir.MatmulPerfMode.DoubleRow,
                    )
                else:
                    nc.tensor.matmul(
                        pt_warm, w_warm[:, 0], rhs_w[:, 0],
                        start=(i == 0), stop=(i == N_WARM - 1),
                    )

            wt = const_pool.tile([128, 3, 4, 2, 2, 128], dt_in)
            # per-image d tiles: [ki, c, pos, h*tc]
            dts = [
                const_pool.tile([128, 2, 4, DPIX], dt_in, name=f"dt{n}", tag=f"dt{n}")
                for n in range(IMGS)
            ]
            # First MM group needs: wt oc0 (all kh/pos) + dt0 rows 0..19.
            # Split the critical bytes across both rings, then order the rest
            # by first use.
            LEAD = 20 * TC  # rows 0..19
            nc.sync.dma_start(out=wt[:, :, :, 0], in_=w_d[:, :, :, 0])
            nc.sync.dma_start(out=dts[0][:, 0, :, 0:LEAD], in_=d_d[:, 0, :, 0, 0:LEAD])
            nc.scalar.dma_start(out=dts[0][:, 1, :, 0:LEAD], in_=d_d[:, 1, :, 0, 0:LEAD])
            nc.scalar.dma_start(out=wt[:, :, :, 1], in_=w_d[:, :, :, 1])
            for c in range(2):
                nc.sync.dma_start(
                    out=dts[0][:, c, :, LEAD:], in_=d_d[:, c, :, 0, LEAD:]
                )
            # remaining images: interleave rings, ordered by first use
            for n in range(1, IMGS):
                nc.sync.dma_start(out=dts[n][:, 0], in_=d_d[:, 0, :, n])
                nc.scalar.dma_start(out=dts[n][:, 1], in_=d_d[:, 1, :, n])

            n_group = 0
            N_GROUPS = IMGS * 2 * N_CHUNKS
            for n in range(IMGS):
                for oc in range(2):
                    for ch in range(N_CHUNKS):
                        h0 = ch * ROWS_PER_CHUNK
                        # pos order (1,2,0,3): M1/M2 finish first so the ACT
                        # copies + GPSIMD subtract run under the remaining
                        # MMs; after the last MM only one DVE op remains.
                        pts = [None] * 4
                        for pos in (1, 2, 0, 3):
                            pt = psum_pool.tile([128, NT], fp32, tag="pt")
                            pts[pos] = pt
                            for kh in range(3):
                                off = (h0 + kh) * TC
                                # winograd rhs windows are fully contiguous:
                                # flat 486-elem inner dim streams best
                                if mode == "fp8dr":
                                    rhs = dts[n][:, :, pos, off : off + NT]
                                    nc.tensor.matmul(
                                        pt,
                                        wt[:, kh, pos, oc],
                                        rhs,
                                        start=(kh == 0),
                                        stop=(kh == 2),
                                        perf_mode=mybir.MatmulPerfMode.DoubleRow,
                                    )
                                else:
                                    for c in range(2):
                                        rhs = dts[n][:, c, pos, off : off + NT]
                                        nc.tensor.matmul(
                                            pt,
                                            wt[:, kh, pos, oc, c],
                                            rhs,
                                            start=(kh == 0 and c == 0),
                                            stop=(kh == 2 and c == 1),
                                        )
                        # output transform: even cols = M0+M1+M2,
                        # odd cols = M1-M2-M3. PSUM has one DVE read port,
                        # so ACT stages M1/M2 into SBUF, GPSIMD (no PSUM
                        # access) takes the SBUF-only subtract, and each
                        # DVE op reads exactly one PSUM tile.
                        ot = out_pool.tile([128, ROWS_PER_CHUNK, OW], dtb)
                        t1 = tmp_pool.tile([128, NT], fp32)
                        t2 = tmp_pool.tile([128, NT], fp32)
                        a = tmp_pool.tile([128, NT], fp32)
                        s = tmp_pool.tile([128, NT], fp32)
                        nc.scalar.copy(t1, pts[1])
                        nc.scalar.copy(t2, pts[2])
                        nc.gpsimd.tensor_tensor(s, t1, t2, mybir.AluOpType.subtract)
                        nc.vector.tensor_tensor(a, pts[0], t1, mybir.AluOpType.add)
                        nc.vector.tensor_tensor(
                            ot[:, :, 0::2].rearrange("p h w -> p (h w)"),
                            a, t2, mybir.AluOpType.add,
                        )
                        nc.vector.tensor_tensor(
                            ot[:, :, 1::2].rearrange("p h w -> p (h w)"),
                            s, pts[3], mybir.AluOpType.subtract,
                        )
                        n_group += 1
                        last = n_group == N_GROUPS
                        if last:
                            # split the final store across both rings so the
                            # completion tail is short
                            s = ROWS_PER_CHUNK // 2
                            nc.sync.dma_start(
                                out=out_d[n, oc * 128 : (oc + 1) * 128, h0 : h0 + s, :],
                                in_=ot[:, 0:s, :],
                            )
                            nc.scalar.dma_start(
                                out=out_d[n, oc * 128 : (oc + 1) * 128,
                                          h0 + s : h0 + ROWS_PER_CHUNK, :],
                                in_=ot[:, s:, :],
                            )
                        else:
                            ring = nc.sync if (n_group % 2 == 0) else nc.scalar
                            ring.dma_start(
                                out=out_d[n, oc * 128 : (oc + 1) * 128,
                                          h0 : h0 + ROWS_PER_CHUNK, :],
                                in_=ot,
                            )
    nc.compile()
    return nc


def get_program(mode="fp8dr"):
    if mode not in _PROGRAM_CACHE:
        _PROGRAM_CACHE[mode] = _build_program(mode)
    return _PROGRAM_CACHE[mode]


def _np_dtype(mode):
    return ml_dtypes.float8_e4m3 if mode == "fp8dr" else ml_dtypes.bfloat16


def prep_weight(weight, mode="fp8dr"):
    """weight [256,256,3,3] OIHW fp32 -> w_sb [128 ki, 3 kh, 4 pos, 2 oc, 2 c, 128 m].

    w1[o,i,kh,pos] = sum_kw G[pos,kw] w[o,i,kh,kw], G = F(2,3) weight transform.
    """
    G = np.array([[1, 0, 0], [0.5, 0.5, 0.5], [0.5, -0.5, 0.5], [0, 0, 1]], np.float32)
    wq = weight.astype(np.int32).astype(np.float32)
    w1 = np.einsum("pk,oihk->oihp", G, wq)  # [o, i, kh, pos]
    w1 = w1.reshape(2, 128, 2, 128, 3, 4)  # [oc, m, c, ki, kh, pos]
    w_sb = np.ascontiguousarray(w1.transpose(3, 4, 5, 0, 2, 1))  # [ki, kh, pos, oc, c, m]
    return w_sb.astype(_np_dtype(mode))


def prep_x_core(x_core, mode="fp8dr"):
    """x_core [IMGS, 256, 56, 56] int32 -> d_sb [128 ki, 2 c, 4 pos, IMGS, 56*27]."""
    xq = np.clip(x_core.astype(np.int32), 0, 7).astype(np.float32)
    xq = xq.reshape(IMGS, 2, 128, H, W)  # [n, c, ki, h, w]
    d0 = xq[..., 0:54:2] - xq[..., 2:56:2]
    d1 = xq[..., 1:55:2] + xq[..., 2:56:2]
    d2 = xq[..., 2:56:2] - xq[..., 1:55:2]
    d3 = xq[..., 1:55:2] - xq[..., 3:56:2]
    d = np.stack([d0, d1, d2, d3], axis=0)  # [pos, n, c, ki, h, tc]
    d_sb = np.ascontiguousarray(d.transpose(3, 2, 0, 1, 4, 5))  # [ki, c, pos, n, h, tc]
    return d_sb.reshape(128, 2, 4, IMGS, DPIX).astype(_np_dtype(mode))


def make_in_maps(x, weight, mode="fp8dr"):
    w_sb = prep_weight(weight, mode)
    return [
        {"x_sb": prep_x_core(x[c * IMGS : (c + 1) * IMGS], mode), "w_sb": w_sb}
        for c in range(N_CORES)
    ]


def kernel(x, weight):
    import time

    from concourse.bass_utils import run_bass_kernel_spmd

    mode = "fp8dr"
    nc = get_program(mode)
    in_maps = make_in_maps(np.asarray(x), np.asarray(weight), mode)
    last_err = None
    for attempt in range(3):
        try:
            res = run_bass_kernel_spmd(nc, in_maps, list(range(N_CORES)))
            break
        except Exception as e:  # transient NRT_EXEC_UNIT_UNRECOVERABLE flakes
            last_err = e
            time.sleep(2.0)
    else:
        raise last_err
    return np.concatenate(
        [res.results[c]["out"] for c in range(N_CORES)], axis=0
    ).astype(np.float32)
